# revision 6
# baseline (speedup 1.0000x reference)
"""GATv2 encoder (2 layers) on 8 TRN2 NeuronCores.

Sharding: edges sorted by dst, dst-range sharded across cores (6250 nodes
per core, 49 tiles of 128). Per node-tile, edges are gathered (dma_gather,
int16 split-table), softmax weights computed edge-major, and scattered into
a PSUM accumulator via one-hot matmuls. h1 is AllGathered (transposed
layout) between the layers.
"""
import numpy as np
import concourse.bass as bass
import concourse.mybir as mybir
import concourse.tile as tile
import concourse.bacc as bacc
from concourse.bass_utils import run_bass_kernel_spmd
from concourse.masks import make_identity

N = 50000
E = 800000
D = 128
H = 2
C1 = 64
C2 = 32
NEG = 0.2
NCORES = 8
NPC = N // NCORES          # 6250 nodes per core
T = 49                     # local tiles of 128 (6272 padded)
NPAD = T * 128             # 6272
GT = NCORES * T            # 392 global tiles
NGPAD = GT * 128           # 50176
SSTAR = 32658              # src split: src2row(32657) = 32767
P = 128

f16 = mybir.dt.float16
f32 = mybir.dt.float32
i16 = mybir.dt.int16

_CACHE = {}


def _build(UA, UB):
    m_a, m_b = UA // P, UB // P
    M = UA + UB
    m = M // P

    nc = bacc.Bacc("TRN2", target_bir_lowering=False, debug=False,
                   num_devices=NCORES, dynamic_dma_scratch_size=65536,
                   num_swdge_queues=4)

    # ---- inputs (shared unless noted) ----
    xt_tiles = nc.dram_tensor("xt_tiles", [GT, P, P], f16, kind="ExternalInput")
    xt_local = nc.dram_tensor("xt_local", [T, P, P], f16, kind="ExternalInput")  # per-core
    w1cat = nc.dram_tensor("w1cat", [P, 2 * P], f16, kind="ExternalInput")
    w2cat = nc.dram_tensor("w2cat", [P, 2 * P], f16, kind="ExternalInput")
    attb1 = nc.dram_tensor("attb1", [P, P], f16, kind="ExternalInput")
    attb2 = nc.dram_tensor("attb2", [P, P], f16, kind="ExternalInput")
    iota_in = nc.dram_tensor("iota", [P, P], f16, kind="ExternalInput")
    b1r_in = nc.dram_tensor("b1r", [P, P], f32, kind="ExternalInput")
    b2r_in = nc.dram_tensor("b2r", [P, C2], f32, kind="ExternalInput")
    # per-core index data
    l1a_in = nc.dram_tensor("l1a", [T, P, UA // 16], i16, kind="ExternalInput")
    l1b_in = nc.dram_tensor("l1b", [T, P, UB // 16], i16, kind="ExternalInput")
    l2a_in = nc.dram_tensor("l2a", [T, P, UA // 16], i16, kind="ExternalInput")
    l2b_in = nc.dram_tensor("l2b", [T, P, UB // 16], i16, kind="ExternalInput")
    lr_in = nc.dram_tensor("lr", [T, P, M // 16], i16, kind="ExternalInput")
    dstrel_in = nc.dram_tensor("dstrel", [T, P, m], f16, kind="ExternalInput")

    out_d = nc.dram_tensor("out", [NPAD, C2], f32, kind="ExternalOutput")

    # ---- internal DRAM ----
    xl1_d = nc.dram_tensor("xl1_d", [NGPAD, P], f16)
    xr1_d = nc.dram_tensor("xr1_d", [NPAD, P], f16)
    h1t_d = nc.dram_tensor("h1t_d", [T * P, P], f16)
    h1t_full = nc.dram_tensor("h1t_full", [GT * P, P], f16, addr_space="Shared")
    xl2_d = nc.dram_tensor("xl2_d", [NGPAD, P], f16)
    xr2_d = nc.dram_tensor("xr2_d", [NPAD, P], f16)

    def dep(a, b, why="fence"):
        tile.add_dep_helper(a.ins, b.ins, sync=True, reason=why)

    with tile.TileContext(nc) as tc:
        with (
            tc.tile_pool(name="cst", bufs=1) as cst,
            tc.tile_pool(name="tfm", bufs=3) as tfm,
            tc.tile_pool(name="edg", bufs=2) as edg,
            tc.tile_pool(name="pp", bufs=2) as pp,
            tc.tile_pool(name="tps", bufs=2, space="PSUM") as tps,
            tc.tile_pool(name="aps", bufs=2, space="PSUM") as aps,
        ):
            # constants
            w1c = cst.tile([P, 2 * P], f16)
            nc.sync.dma_start(out=w1c[:], in_=w1cat[:, :])
            w2c = cst.tile([P, 2 * P], f16)
            nc.sync.dma_start(out=w2c[:], in_=w2cat[:, :])
            ab1 = cst.tile([P, P], f16)
            nc.sync.dma_start(out=ab1[:], in_=attb1[:, :])
            ab2 = cst.tile([P, P], f16)
            nc.sync.dma_start(out=ab2[:], in_=attb2[:, :])
            iot = cst.tile([P, P], f16)
            nc.sync.dma_start(out=iot[:], in_=iota_in[:, :])
            b1t = cst.tile([P, P], f32)
            nc.sync.dma_start(out=b1t[:], in_=b1r_in[:, :])
            b2t = cst.tile([P, C2], f32)
            nc.sync.dma_start(out=b2t[:], in_=b2r_in[:, :])
            id32 = cst.tile([P, P], f32)
            make_identity(nc, id32[:])

            # ---------- transform phase helper ----------
            def transform(src_dram, n_tiles, rhs_ap, ncols, outs, fences, par):
                """out tiles [P, ncols] fp16 <- (src tile)^T-major matmul."""
                for t in range(n_tiles):
                    xt = tfm.tile([P, P], f16, tag="xt")
                    nc.sync.dma_start(out=xt[:], in_=src_dram(t))
                    ps = tps.tile([P, 2 * P], f32, tag="tps", space="PSUM")
                    nc.tensor.matmul(out=ps[:, :ncols], lhsT=xt[:], rhs=rhs_ap,
                                     start=True, stop=True)
                    s16 = tfm.tile([P, 2 * P], f16, tag="s16")
                    if (t + par) % 2 == 0:
                        nc.vector.tensor_copy(out=s16[:, :ncols], in_=ps[:, :ncols])
                    else:
                        nc.scalar.copy(out=s16[:, :ncols], in_=ps[:, :ncols])
                    for dst_ap, c0, c1 in outs(t):
                        wr = nc.sync.dma_start(out=dst_ap, in_=s16[:, c0:c1])
                        fences.append(wr)

            # ---------- L1 transforms ----------
            f1 = []
            transform(lambda t: xt_tiles[t, :, :], GT, w1c[:, 0:P], P,
                      lambda t: [(xl1_d[t * P:(t + 1) * P, :], 0, P)], f1, 0)
            transform(lambda t: xt_local[t, :, :], T, w1c[:, P:2 * P], P,
                      lambda t: [(xr1_d[t * P:(t + 1) * P, :], 0, P)], f1, 1)
            mk1 = cst.tile([1, 1], f32, tag="mk1")
            fence1 = nc.vector.memset(mk1[:], 0.0)
            for wi in f1:
                dep(fence1, wi)

            # ---------- edge phase helper ----------
            def edge_layer(lt, xla, xlb, xrl, ia_in, ib_in, fence, attb, nheads,
                           outcols, post):
                ia = edg.tile([P, UA // 16], i16, tag="ia")
                nc.sync.dma_start(out=ia[:], in_=ia_in[lt, :, :])
                ib = edg.tile([P, UB // 16], i16, tag="ib")
                nc.sync.dma_start(out=ib[:], in_=ib_in[lt, :, :])
                ir = edg.tile([P, M // 16], i16, tag="ir")
                nc.sync.dma_start(out=ir[:], in_=lr_in[lt, :, :])
                drl = edg.tile([P, m], f16, tag="drl")
                nc.sync.dma_start(out=drl[:], in_=dstrel_in[lt, :, :])

                msg = edg.tile([P, m, P], f16, tag="msg")
                xrd = edg.tile([P, m, P], f16, tag="xrd")
                GCH = 768
                qn = [0]

                def gath(dst_tile, c0, table, idx_t, nidx):
                    for off in range(0, nidx, GCH):
                        n = min(GCH, nidx - off)
                        g = nc.gpsimd.dma_gather(
                            dst_tile[:, c0 + off // P:c0 + (off + n) // P, :],
                            table,
                            idx_t[:, off // 16:(off + n) // 16],
                            n, n, P, queue_num=qn[0] % 4)
                        qn[0] += 1
                        dep(g, fence)

                gath(msg, 0, xla, ia, UA)
                gath(msg, m_a, xlb, ib, UB)
                gath(xrd, 0, xrl, ir, M)

                es = edg.tile([P, m, P], f16, tag="es")
                nc.vector.tensor_tensor(out=es[:], in0=msg[:], in1=xrd[:],
                                        op=mybir.AluOpType.add)
                ee = edg.tile([P, m, P], f16, tag="ee")
                nc.scalar.activation(out=ee[:], in_=es[:],
                                     func=mybir.ActivationFunctionType.Prelu,
                                     alpha=NEG)
                ch = P // nheads
                tt = edg.tile([P, m, P], f16, tag="tt")
                lg = edg.tile([P, m * nheads], f32, tag="lg")
                for j in range(m):
                    nc.vector.tensor_tensor(out=tt[:, j, :], in0=ee[:, j, :],
                                            in1=attb[:],
                                            op=mybir.AluOpType.mult)
                    if nheads > 1:
                        nc.vector.tensor_reduce(
                            out=lg[:, j * nheads:(j + 1) * nheads],
                            in_=bass.AP(tt.tensor, tt[:].offset + j * P,
                                        [tt[:].ap[0], [ch, nheads], [1, ch]]),
                            axis=mybir.AxisListType.X, op=mybir.AluOpType.add)
                    else:
                        nc.vector.tensor_reduce(
                            out=lg[:, j:j + 1],
                            in_=tt[:, j, :],
                            axis=mybir.AxisListType.X, op=mybir.AluOpType.add)
                w32 = edg.tile([P, m * nheads], f32, tag="w32")
                nc.scalar.activation(out=w32[:], in_=lg[:],
                                     func=mybir.ActivationFunctionType.Exp)
                w16 = edg.tile([P, m * nheads], f16, tag="w16")
                nc.vector.tensor_copy(out=w16[:], in_=w32[:])

                rw = outcols  # rhs width = payload + nheads
                rhs = edg.tile([P, m, rw], f16, tag="rhs")
                pay = rw - nheads
                chp = pay // nheads
                for j in range(m):
                    for h in range(nheads):
                        nc.vector.tensor_scalar_mul(
                            out=rhs[:, j, h * chp:(h + 1) * chp],
                            in0=msg[:, j, h * ch:h * ch + chp],
                            scalar1=w32[:, j * nheads + h:j * nheads + h + 1])
                    nc.vector.tensor_copy(
                        out=rhs[:, j, pay:rw],
                        in_=w16[:, j * nheads:(j + 1) * nheads])

                oh = edg.tile([P, m, P], f16, tag="oh")
                for j in range(m):
                    nc.vector.tensor_tensor(
                        out=oh[:, j, :],
                        in0=drl[:, j:j + 1].to_broadcast([P, P]),
                        in1=iot[:],
                        op=mybir.AluOpType.is_equal)

                acc = aps.tile([P, rw], f32, tag="acc", space="PSUM")
                for j in range(m):
                    nc.tensor.matmul(out=acc[:], lhsT=oh[:, j, :],
                                     rhs=rhs[:, j, :],
                                     start=(j == 0), stop=(j == m - 1))
                post(lt, acc)

            # ---------- L1 edge phase ----------
            f_ag = []

            def post1(lt, acc):
                rs = pp.tile([P, H], f32, tag="rs")
                nc.vector.reciprocal(out=rs[:], in_=acc[:, P:P + H])
                hv = pp.tile([P, P], f32, tag="hv")
                for h in range(H):
                    nc.vector.tensor_scalar_mul(
                        out=hv[:, h * C1:(h + 1) * C1],
                        in0=acc[:, h * C1:(h + 1) * C1],
                        scalar1=rs[:, h:h + 1])
                nc.vector.tensor_tensor(out=hv[:], in0=hv[:], in1=b1t[:],
                                        op=mybir.AluOpType.add)
                # elu(x) = relu(x) + exp(min(x,0)) - 1
                mn = pp.tile([P, P], f32, tag="mn")
                nc.vector.tensor_scalar_min(out=mn[:], in0=hv[:], scalar1=0.0)
                ex = pp.tile([P, P], f32, tag="ex")
                nc.scalar.activation(out=ex[:], in_=mn[:],
                                     func=mybir.ActivationFunctionType.Exp)
                rl = pp.tile([P, P], f32, tag="rl")
                nc.scalar.activation(out=rl[:], in_=hv[:],
                                     func=mybir.ActivationFunctionType.Relu)
                h1f = pp.tile([P, P], f32, tag="h1f")
                nc.vector.tensor_tensor(out=h1f[:], in0=rl[:], in1=ex[:],
                                        op=mybir.AluOpType.add)
                nc.vector.tensor_scalar_add(out=h1f[:], in0=h1f[:], scalar1=-1.0)
                tp = tps.tile([P, P], f32, tag="tp", space="PSUM")
                nc.tensor.transpose(out=tp[:], in_=h1f[:], identity=id32[:])
                h1t16 = pp.tile([P, P], f16, tag="h1t16")
                nc.scalar.copy(out=h1t16[:], in_=tp[:])
                wr = nc.sync.dma_start(out=h1t_d[lt * P:(lt + 1) * P, :],
                                       in_=h1t16[:])
                f_ag.append(wr)

            for lt in range(T):
                edge_layer(lt, xl1_d[:SSTAR, :], xl1_d[SSTAR:, :], xr1_d[:, :],
                           l1a_in, l1b_in, fence1, ab1, H, P + H, post1)

            # ---------- AllGather ----------
            cc = nc.gpsimd.collective_compute(
                "AllGather", mybir.AluOpType.bypass,
                replica_groups=[list(range(NCORES))],
                ins=[h1t_d[:, :]], outs=[h1t_full[:, :]])
            for wi in f_ag:
                dep(cc, wi, "ag-in")

            # ---------- L2 transforms ----------
            f2 = []

            def src_full(t):
                return h1t_full[t * P:(t + 1) * P, :]

            def src_loc(t):
                return h1t_d[t * P:(t + 1) * P, :]

            t2_reads = []
            for t in range(GT):
                xt = tfm.tile([P, P], f16, tag="xt")
                rd = nc.sync.dma_start(out=xt[:], in_=src_full(t))
                dep(rd, cc, "ag-out")
                ps = tps.tile([P, 2 * P], f32, tag="tps", space="PSUM")
                nc.tensor.matmul(out=ps[:, :P], lhsT=xt[:], rhs=w2c[:, 0:P],
                                 start=True, stop=True)
                s16 = tfm.tile([P, 2 * P], f16, tag="s16")
                if t % 2 == 0:
                    nc.vector.tensor_copy(out=s16[:, :P], in_=ps[:, :P])
                else:
                    nc.scalar.copy(out=s16[:, :P], in_=ps[:, :P])
                f2.append(nc.sync.dma_start(out=xl2_d[t * P:(t + 1) * P, :],
                                            in_=s16[:, 0:P]))
            for t in range(T):
                xt = tfm.tile([P, P], f16, tag="xt")
                rd = nc.sync.dma_start(out=xt[:], in_=src_loc(t))
                dep(rd, cc, "loc-after-ag")
                ps = tps.tile([P, 2 * P], f32, tag="tps", space="PSUM")
                nc.tensor.matmul(out=ps[:, :P], lhsT=xt[:], rhs=w2c[:, P:2 * P],
                                 start=True, stop=True)
                s16 = tfm.tile([P, 2 * P], f16, tag="s16")
                nc.scalar.copy(out=s16[:, :P], in_=ps[:, :P])
                f2.append(nc.sync.dma_start(out=xr2_d[t * P:(t + 1) * P, :],
                                            in_=s16[:, 0:P]))
            mk2 = cst.tile([1, 1], f32, tag="mk2")
            fence2 = nc.vector.memset(mk2[:], 0.0)
            for wi in f2:
                dep(fence2, wi)

            # ---------- L2 edge phase ----------
            def post2(lt, acc):
                rs = pp.tile([P, H], f32, tag="rs")
                nc.vector.reciprocal(out=rs[:, 0:1], in_=acc[:, C2:C2 + 1])
                ov = pp.tile([P, C2], f32, tag="ov")
                nc.vector.tensor_scalar_mul(out=ov[:], in0=acc[:, 0:C2],
                                            scalar1=rs[:, 0:1])
                nc.vector.tensor_tensor(out=ov[:], in0=ov[:], in1=b2t[:],
                                        op=mybir.AluOpType.add)
                nc.sync.dma_start(out=out_d[lt * P:(lt + 1) * P, :], in_=ov[:])

            for lt in range(T):
                edge_layer(lt, xl2_d[:32768, :], xl2_d[32768:, :], xr2_d[:, :],
                           l2a_in, l2b_in, fence2, ab2, 1, C2 + 1, post2)

    nc.compile()
    return nc


def _wrap_idx(vals, nslots):
    """[nslots] -> [128, nslots//16] int16 (idx j at [j%16, j//16], x8)."""
    arr = np.zeros((16, nslots // 16), np.int16)
    arr[np.arange(nslots) % 16, np.arange(nslots) // 16] = vals
    return np.tile(arr, (8, 1))


def kernel(x, edge_index, Wl1, Wr1, att1, b1, Wl2, Wr2, att2, b2):
    x = np.asarray(x, np.float32)
    ei = np.asarray(edge_index)
    src = np.concatenate([ei[0], np.arange(N, dtype=np.int64)]).astype(np.int64)
    dst = np.concatenate([ei[1], np.arange(N, dtype=np.int64)]).astype(np.int64)

    core = dst // NPC
    rem = dst % NPC
    tl = rem // P
    zone = (src >= SSTAR).astype(np.int64)
    order = np.lexsort((zone, tl, core))
    s_s, d_rem, d_tl, d_core, s_zone = (src[order], rem[order], tl[order],
                                        core[order], zone[order])

    key = d_core * (T * 2) + d_tl * 2 + s_zone
    bounds = np.searchsorted(key, np.arange(NCORES * T * 2 + 1))
    cntA = (bounds[1::2] - bounds[0:-1:2]).reshape(NCORES, T)
    cntB = (bounds[2::2] - bounds[1::2]).reshape(NCORES, T)
    UA = max(int(-(-cntA.max() // P) * P), P)
    UB = max(int(-(-cntB.max() // P) * P), P)
    M = UA + UB
    m = M // P

    if (UA, UB) not in _CACHE:
        _CACHE[(UA, UB)] = _build(UA, UB)
    nc = _CACHE[(UA, UB)]

    # shared tensors
    xg = np.zeros((NGPAD, D), np.float32)
    xg[:N] = x
    xt_tiles = np.ascontiguousarray(
        xg.T.reshape(D, GT, P).transpose(1, 0, 2)).astype(np.float16)
    w1cat = np.concatenate([Wl1, Wr1], 1).astype(np.float16)
    w2cat = np.zeros((P, 2 * P), np.float16)
    w2cat[:, :C2] = np.asarray(Wl2, np.float32).astype(np.float16)
    w2cat[:, P:P + C2] = np.asarray(Wr2, np.float32).astype(np.float16)
    attb1 = np.broadcast_to(np.asarray(att1, np.float16).reshape(1, P),
                            (P, P)).copy()
    ab2 = np.zeros((P, P), np.float16)
    ab2[:, :C2] = np.asarray(att2, np.float32).reshape(1, C2)
    iota = np.broadcast_to(np.arange(P, dtype=np.float16), (P, P)).copy()
    b1r = np.broadcast_to(np.asarray(b1, np.float32).reshape(1, P),
                          (P, P)).copy()
    b2r = np.broadcast_to(np.asarray(b2, np.float32).reshape(1, C2),
                          (P, C2)).copy()

    src2row = (s_s // NPC) * NPAD + (s_s % NPC)

    in_maps = []
    for c in range(NCORES):
        l1a = np.zeros((T, P, UA // 16), np.int16)
        l1b = np.zeros((T, P, UB // 16), np.int16)
        l2a = np.zeros((T, P, UA // 16), np.int16)
        l2b = np.zeros((T, P, UB // 16), np.int16)
        lr = np.zeros((T, P, M // 16), np.int16)
        drel = np.full((T, P, m), -1.0, np.float16)
        for t in range(T):
            gi = (c * T + t) * 2
            a0, a1, b1e = bounds[gi], bounds[gi + 1], bounds[gi + 2]
            nA, nB = a1 - a0, b1e - a1
            sA = np.zeros(UA, np.int64)
            sA[:nA] = s_s[a0:a1]
            r2A = np.zeros(UA, np.int64)
            r2A[:nA] = src2row[a0:a1]
            sB = np.full(UB, N - 1, np.int64)
            sB[:nB] = s_s[a1:b1e]
            r2B = np.full(UB, (N - 1) // NPC * NPAD + (N - 1) % NPC, np.int64)
            r2B[:nB] = src2row[a1:b1e]
            l1a[t] = _wrap_idx(sA, UA)
            l1b[t] = _wrap_idx(sB - SSTAR, UB)
            l2a[t] = _wrap_idx(r2A, UA)
            l2b[t] = _wrap_idx(r2B - 32768, UB)
            dloc = np.zeros(M, np.int64)
            dloc[:nA] = d_rem[a0:a1]
            dloc[UA:UA + nB] = d_rem[a1:b1e]
            lr[t] = _wrap_idx(dloc, M)
            dr = np.full(M, -1.0, np.float32)
            dr[:nA] = d_rem[a0:a1] - t * P
            dr[UA:UA + nB] = d_rem[a1:b1e] - t * P
            drel[t][np.arange(M) % P, np.arange(M) // P] = dr.astype(np.float16)
        xl = np.zeros((T * P, D), np.float32)
        lo, hi = c * NPC, (c + 1) * NPC
        xl[:NPC] = x[lo:hi]
        xt_local = np.ascontiguousarray(
            xl.T.reshape(D, T, P).transpose(1, 0, 2)).astype(np.float16)
        in_maps.append({
            "xt_tiles": xt_tiles, "xt_local": xt_local,
            "w1cat": w1cat, "w2cat": w2cat, "attb1": attb1, "attb2": ab2,
            "iota": iota, "b1r": b1r, "b2r": b2r,
            "l1a": l1a, "l1b": l1b, "l2a": l2a, "l2b": l2b,
            "lr": lr, "dstrel": drel,
        })

    res = run_bass_kernel_spmd(nc, in_maps, core_ids=list(range(NCORES)))
    kernel.last_results = res
    out = np.concatenate([res.results[c]["out"][:NPC] for c in range(NCORES)], 0)
    return out.astype(np.float32)
